# revision 14
# baseline (speedup 1.0000x reference)
"""Ring-attention (context-parallel) kernel for 8 TRN2 NeuronCores.

Problem: x_q [1,2048,2048], x_kv [1,8192,2048], GQA attention (16 q heads,
4 kv heads, D=128) where q occupies global positions 6144..8191 of the
8192-long key sequence (causal on the last 2048 block, full attention on
the first 6144 keys), followed by an output projection.

Strategy (sequence/context parallel, the module's native layout):
  - q rows are split into 16 strips of 128; core c owns strips {c, 15-c}
    (folded pairing -> every core attends to the same total number of keys,
    perfectly balancing the causal wedge).
  - x_kv is sequence-sharded 8 x 1024 rows; each core projects its local
    K/V shard to K^T / V (bf16); per head group one AllGather for K^T and
    one for V share the full tensors. Phase A runs K_0 and the first half
    of V concurrently against the input DMA (disjoint PSUM banks), so the
    first collective fires ~1 matmul-round after the inputs land, and the
    8 collectives stream behind each other under the early attention
    groups.
  - Projection weights are replicated (bf16).
  - Each core computes all 16 heads for its 256 q rows, then the full
    output projection for those rows -> no cross-core reduction at the end.

Engine schedule: the attention inner loop is paced by the Scalar engine
(exp of the scores, ~1.2us per 128x1024 chunk). The other engines are kept
out of its way by pipelining at three levels:
  - chunk level: the PV matmuls of chunk i are emitted after the S matmuls
    of chunk i+1, so the PE never head-of-line blocks on the current exp;
  - group level: the chunk pipeline runs *across* group boundaries, and
    the normalization + Wo-fold tail of group g is chopped into small
    items pumped through group g+1's chunk loop (a serial tail at the
    boundary also let the PE clock-gate re-throttle to 1.2 GHz);
  - phase level: only the first 4 q heads are projected before attention;
    the other 12 heads' Q projection runs as pumped items inside group
    0's loop (they are first needed by group 1).
DMA descriptor count is minimized (the sync queue costs ~0.7us per
descriptor regardless of size): batched input loads, a resident wq tile.
All PSUM evacuations go to the Vector engine; a third of the softmax
denominator accumulation goes to the otherwise-idle GpSimd engine. The
softmax denominator is broadcast to all partitions with an all-ones
128x128 matmul so the reciprocal runs full-width on the Vector engine
(a [1,512] single-partition reciprocal costs ~3.3us; [128,512] is 0.6us).
"""

import numpy as np
import ml_dtypes

import concourse.bass as bass
import concourse.mybir as mybir
import concourse.tile as tile
from concourse import bacc, bass_utils

BF16 = ml_dtypes.bfloat16
F32 = mybir.dt.float32
BF = mybir.dt.bfloat16

N_CORES = 8
H = 16          # query heads
HKV = 4         # kv heads
D = 128         # head dim
HID = H * D     # 2048
SL = 2048       # q rows (global)
SKV = 8192      # kv rows (global)
QS = 256        # q rows per core (2 strips of 128)
LKV = SKV // N_CORES   # 1024 local kv rows
HC = HID // 128        # 16 hid chunks
KC = SKV // 128        # 64 key chunks
RANK_OFF = SKV - SL    # 6144: global position of q row 0
BND = RANK_OFF // 128  # 48: first key chunk needing a causal mask
SCALE = 1.0 / float(np.sqrt(D))
W = 4 * QS             # 1024: 4 heads x 256 q

_CACHE = {}


def _build():
    nc = bacc.Bacc("TRN2", target_bir_lowering=False, debug=False,
                   num_devices=N_CORES)

    xqT = nc.dram_tensor("xqT", [HID, QS], BF, kind="ExternalInput")
    xkvT = nc.dram_tensor("xkvT", [HID, LKV], BF, kind="ExternalInput")
    wqT = nc.dram_tensor("wqT", [HID, HID], BF, kind="ExternalInput")
    wkT = nc.dram_tensor("wkT", [HID, HKV * D], BF, kind="ExternalInput")
    wvT = nc.dram_tensor("wvT", [HID, HKV * D], BF, kind="ExternalInput")
    woT = nc.dram_tensor("woT", [HID, HID], BF, kind="ExternalInput")
    # boundary causal masks: 16 key chunks (48..63) x [128 keys, 4 heads x 256 q]
    maskD = nc.dram_tensor("mask", [16 * 128, 4 * QS], BF, kind="ExternalInput")
    outT = nc.dram_tensor("outT", [HID, QS], F32, kind="ExternalOutput")

    with tile.TileContext(nc) as tc:
        _body(nc, tc, xqT, xkvT, wqT, wkT, wvT, woT, maskD, outT)
    nc.compile()
    return nc


def _body(nc, tc, xqT, xkvT, wqT, wkT, wvT, woT, maskD, outT):
    from contextlib import ExitStack
    ctx = ExitStack()
    with ctx:
        const = ctx.enter_context(tc.tile_pool(name="const", bufs=1))
        persist = ctx.enter_context(tc.tile_pool(name="persist", bufs=1))
        dram = ctx.enter_context(tc.tile_pool(name="dram", bufs=1, space="DRAM"))

        ones_kk = const.tile([128, 128], BF)
        nc.gpsimd.memset(ones_kk[:], 1.0)
        # preload the ACT exp table during Phase A so the first attention
        # exp doesn't pay the ~2.7us ACT_TABLE_LOAD
        warm = const.tile([1, 2], F32)
        nc.scalar.activation(warm[:, 1:2], warm[:, 0:1],
                             mybir.ActivationFunctionType.Exp)

        qt_sb = persist.tile([128, H, QS], BF)        # Q^T per head
        ao_sb = persist.tile([128, HKV, 4 * QS], BF)  # normalized O^T per g

        xq_sb = persist.tile([128, HC, QS], BF)   # Q^T input, hid-chunked
        nc.gpsimd.dma_start(
            xq_sb[:], xqT.ap().rearrange("(a p) q -> p a q", p=128))

        # per-g bounce + gather tiles, K^T [D, LKV] and V [LKV, D] separate
        # so the K collective (which gates the first attention matmuls)
        # fires without waiting for V
        bncK = [dram.tile([128 * LKV], BF, name=f"bncK{g}", uniquify=False)
                for g in range(HKV)]
        bncV = [dram.tile([128 * LKV], BF, name=f"bncV{g}", uniquify=False)
                for g in range(HKV)]
        gathK = [dram.tile([N_CORES * 128 * LKV], BF, addr_space="Shared",
                           name=f"gathK{g}", uniquify=False)
                 for g in range(HKV)]
        gathV = [dram.tile([N_CORES * 128 * LKV], BF, addr_space="Shared",
                           name=f"gathV{g}", uniquify=False)
                 for g in range(HKV)]
        rg = [list(range(N_CORES))]

        def ag(which, g):
            b, t = (bncK, gathK) if which == "K" else (bncV, gathV)
            nc.gpsimd.collective_compute(
                "AllGather", mybir.AluOpType.bypass, replica_groups=rg,
                ins=[b[g].opt()], outs=[t[g].opt()])

        wqpool = ctx.enter_context(tc.tile_pool(name="wqp", bufs=1))
        wq_sb = wqpool.tile([128, HC, HID], BF)

        # ---------------- Phase A: local K/V projection -------------------
        with tc.tile_pool(name="kva", bufs=1) as kva:
            xkv_sb = kva.tile([128, HC, LKV], BF)
            wk_sb = kva.tile([128, HC, HKV * D], BF)
            wv_sb = kva.tile([128, HC, HKV * D], BF)
            # batched input loads: the sync DMA queue costs ~0.7us per
            # descriptor, so few big transfers beat many small ones
            for h2 in range(2):
                nc.sync.dma_start(
                    wk_sb[:, h2 * 8:(h2 + 1) * 8, :],
                    wkT.ap()[h2 * 1024:(h2 + 1) * 1024, :]
                    .rearrange("(a p) d -> p a d", p=128))
                nc.sync.dma_start(
                    wv_sb[:, h2 * 8:(h2 + 1) * 8, :],
                    wvT.ap()[h2 * 1024:(h2 + 1) * 1024, :]
                    .rearrange("(a p) d -> p a d", p=128))
            for h2 in range(8):
                nc.sync.dma_start(
                    xkv_sb[:, h2 * 2:(h2 + 1) * 2, :],
                    xkvT.ap()[h2 * 256:(h2 + 1) * 256, :]
                    .rearrange("(a p) d -> p a d", p=128))
            # wq resident load follows the phase-A inputs on the queue
            for hc in range(HC):
                nc.sync.dma_start(
                    wq_sb[:, hc, :], wqT.ap()[hc * 128:(hc + 1) * 128, :])

            v_all = kva.tile([128, LKV // 128, HKV * D], BF)

            # K_0 and the first half of V run concurrently (2 + 4 PSUM
            # banks), both paced by the arriving xkv chunks; AG_0K fires
            # right after the last input chunk is consumed.
            with (
                tc.tile_pool(name="k0ps", bufs=1, space="PSUM") as k0ps,
                tc.tile_pool(name="vps", bufs=1, space="PSUM") as vps,
            ):
                ps_k0 = k0ps.tile([128, LKV], F32, tag="kt")
                ps_v0 = vps.tile([128, 4, HKV * D], F32, tag="v")
                for hc in range(HC):
                    for nn in range(0, LKV, 512):
                        nc.tensor.matmul(
                            ps_k0[:, nn:nn + 512], wk_sb[:, hc, 0:D],
                            xkv_sb[:, hc, nn:nn + 512],
                            start=(hc == 0), stop=(hc == HC - 1))
                    for i in range(4):
                        nc.tensor.matmul(
                            ps_v0[:, i, :],
                            xkv_sb[:, hc, i * 128:(i + 1) * 128],
                            wv_sb[:, hc, :],
                            start=(hc == 0), stop=(hc == HC - 1))
                kt_loc = kva.tile([128, LKV], BF, tag="ktloc0")
                nc.vector.tensor_copy(kt_loc[:], ps_k0[:])
                nc.gpsimd.dma_start(
                    bncK[0].rearrange("(p c) -> p c", p=128), kt_loc[:])
                ag("K", 0)
                for i in range(4):
                    nc.vector.tensor_copy(v_all[:, i, :], ps_v0[:, i, :])
                # second half of V (rotates onto the same vps buffer)
                ps_v1 = vps.tile([128, 4, HKV * D], F32, tag="v")
                for hc in range(HC):
                    for i in range(4):
                        nc.tensor.matmul(
                            ps_v1[:, i, :],
                            xkv_sb[:, hc, (4 + i) * 128:(5 + i) * 128],
                            wv_sb[:, hc, :],
                            start=(hc == 0), stop=(hc == HC - 1))
                for i in range(4):
                    nc.vector.tensor_copy(v_all[:, 4 + i, :], ps_v1[:, i, :])
            for g in range(HKV):
                nc.gpsimd.dma_start(
                    bncV[g].rearrange("(a p d) -> p a d", p=128, d=D),
                    v_all[:, :, g * D:(g + 1) * D])
            ag("V", 0)
            with tc.tile_pool(name="kps", bufs=2, space="PSUM") as kps:
                for g in range(1, HKV):
                    ps = kps.tile([128, LKV], F32, tag="kt")
                    for hc in range(HC):
                        lhsT = wk_sb[:, hc, g * D:(g + 1) * D]
                        for nn in range(0, LKV, 512):
                            nc.tensor.matmul(
                                ps[:, nn:nn + 512], lhsT,
                                xkv_sb[:, hc, nn:nn + 512],
                                start=(hc == 0), stop=(hc == HC - 1))
                    kt_loc = kva.tile([128, LKV], BF, tag="ktloc")
                    nc.vector.tensor_copy(kt_loc[:], ps[:])
                    nc.gpsimd.dma_start(
                        bncK[g].rearrange("(p c) -> p c", p=128), kt_loc[:])
                    ag("K", g)
                    ag("V", g)

        # late pool: reuses the SBUF freed by the phase-A tiles
        late = ctx.enter_context(tc.tile_pool(name="late", bufs=1))
        out_acc = late.tile([128, HC, QS], F32)
        # causal-mask load deferred here: first needed at key chunk 48 of
        # group 0, long after attention starts
        mask_sb = late.tile([128, 16, 4 * QS], BF)
        nc.gpsimd.dma_start(
            mask_sb[:], maskD.ap().rearrange("(a p) q -> p a q", p=128))

        # -------- Phase C: Q projection of group-0 heads only -------------
        # (the other 12 heads are projected inside group 0's loop below)
        with tc.tile_pool(name="qps", bufs=2, space="PSUM") as qps:
            for hb in range(2):            # blocks of 2 heads
                ps = qps.tile([128, 2, 512], F32, tag="q")
                for hc in range(HC):
                    for hh in range(2):
                        nc.tensor.matmul(
                            ps[:, hh, 0:QS],
                            wq_sb[:, hc,
                                  hb * 256 + hh * 128:hb * 256 + hh * 128 + 128],
                            xq_sb[:, hc, :],
                            start=(hc == 0), stop=(hc == HC - 1))
                for hh in range(2):
                    nc.vector.tensor_copy(
                        qt_sb[:, hb * 2 + hh, :], ps[:, hh, 0:QS])

        # ---------------- Phase D: attention ------------------------------
        with (
            tc.tile_pool(name="kvstream", bufs=3) as kvstream,
            tc.tile_pool(name="attw", bufs=3) as attw,
            tc.tile_pool(name="rec", bufs=2) as rec,
            tc.tile_pool(name="wop", bufs=2) as wop,
            tc.tile_pool(name="accp", bufs=2) as accp,
            tc.tile_pool(name="stps", bufs=2, space="PSUM") as stps,
            tc.tile_pool(name="otps", bufs=1, space="PSUM") as otps,
            tc.tile_pool(name="tailps", bufs=1, space="PSUM") as tailps,
        ):
            def make_tail(g, acc2, wo_g, pool, tag):
                """Normalization + Wo fold of group g as a list of small
                emission callbacks (pumped through the next group's loop)."""
                def norm(nn):
                    t = pool.tile([128, 2, 512], F32, tag=tag)
                    bc = t[:, 0, :]
                    nc.tensor.matmul(bc, ones_kk[:],
                                     acc2[:, nn:nn + 512],
                                     start=True, stop=False)
                    nc.tensor.matmul(bc, ones_kk[:],
                                     acc2[:, W + nn:W + nn + 512],
                                     start=False, stop=True)
                    recip_f = rec.tile([128, 512], F32, tag="recipf")
                    nc.vector.reciprocal(recip_f[:], bc)
                    recip = rec.tile([128, 512], BF, tag="recip")
                    nc.vector.tensor_copy(recip[:], recip_f[:])
                    nc.vector.tensor_mul(ao_sb[:, g, nn:nn + 512],
                                         ao_sb[:, g, nn:nn + 512], recip[:])

                def fold(jc):
                    # two hid-chunks per item: one accumulation chain per
                    # PSUM bank of the tail tile, one strided evacuation
                    t = pool.tile([128, 2, 512], F32, tag=tag)
                    for j in range(2):
                        fp = t[:, j, 0:QS]
                        for hh in range(4):
                            nc.tensor.matmul(
                                fp,
                                wo_g[:, hh, (jc + j) * 128:(jc + j + 1) * 128],
                                ao_sb[:, g, hh * QS:(hh + 1) * QS],
                                start=(hh == 0), stop=(hh == 3))
                    dst = out_acc[:, jc:jc + 2, :]
                    if g == 0:
                        nc.vector.tensor_copy(dst, t[:, :, 0:QS])
                    else:
                        nc.vector.tensor_add(dst, dst, t[:, :, 0:QS])

                items = [lambda nn=nn: norm(nn) for nn in range(0, W, 512)]
                items += [lambda jc=jc: fold(jc) for jc in range(0, HC, 2)]
                return items

            def make_qproj():
                """Q projection of heads 4..15 as pumped items: one item
                per (head-pair block, hid chunk), plus an evacuation item.
                The psum accumulation tile lives in the tail pool (which
                group tails only need from group 1 onward)."""
                state = {}

                def step(hb, hc):
                    if hc == 0:
                        state[hb] = tailps.tile([128, 2, 512], F32,
                                                tag="tail", name=f"qbg{hb}")
                    ps = state[hb]
                    for hh in range(2):
                        nc.tensor.matmul(
                            ps[:, hh, 0:QS],
                            wq_sb[:, hc,
                                  hb * 256 + hh * 128:
                                  hb * 256 + hh * 128 + 128],
                            xq_sb[:, hc, :],
                            start=(hc == 0), stop=(hc == HC - 1))

                def evac(hb):
                    ps = state.pop(hb)
                    for hh in range(2):
                        nc.vector.tensor_copy(
                            qt_sb[:, hb * 2 + hh, :], ps[:, hh, 0:QS])

                items = []
                for hb in range(2, 8):
                    items += [lambda hb=hb, hc=hc: step(hb, hc)
                              for hc in range(HC)]
                    items.append(lambda hb=hb: evac(hb))
                return items

            # ---- globally software-pipelined chunk loop: S+exp for chunk
            # i is emitted first, then PV + denominator-add for chunk i-1
            # (possibly from the previous group), so neither the PE nor
            # the ACT engine ever stalls at a group boundary.
            pend = []      # [(ex, l, v_slab, kc, g, ot_ps, acc2, wo_g)]
            bg = make_qproj()
            bg_deadline = [KC]     # chunk index by which bg must drain

            def flush():
                ex, l, v_slab, kc, eg, ot_ps, acc2, wo_g = pend.pop(0)
                if kc >= 56:
                    for hh in range(4):
                        nc.tensor.matmul(
                            ot_ps[:, hh * QS + 128:(hh + 1) * QS],
                            v_slab[:, l, :],
                            ex[:, hh * QS + 128:(hh + 1) * QS],
                            start=(kc == 0), stop=(kc == KC - 1))
                else:
                    for nn in range(0, W, 512):
                        nc.tensor.matmul(
                            ot_ps[:, nn:nn + 512],
                            v_slab[:, l, :],
                            ex[:, nn:nn + 512],
                            start=(kc == 0), stop=(kc == KC - 1))
                # denominator accumulation; every 3rd add runs on the
                # otherwise-idle gpsimd engine so the vector engine keeps
                # pace with the scalar engine's exps
                half = (kc % 2) * W
                dst = acc2[:, half:half + W]
                if kc < 2:
                    nc.vector.tensor_copy(dst, ex[:, 0:W])
                elif kc % 3 == 2:
                    nc.gpsimd.tensor_add(dst, dst, ex[:, 0:W])
                else:
                    nc.vector.tensor_add(dst, dst, ex[:, 0:W])
                if kc == KC - 1:
                    # group finished: free ot_ps, defer the tail
                    nc.vector.tensor_copy(ao_sb[:, eg, :], ot_ps[:])
                    if eg < HKV - 1:
                        bg.extend(make_tail(eg, acc2, wo_g, tailps, "tail"))
                        bg_deadline[0] = (eg + 2) * KC
                    else:
                        for item in make_tail(eg, acc2, wo_g, stps, "st"):
                            item()

            for g in range(HKV):
                ot_ps = otps.tile([128, W], F32, tag="ot")
                # bf16 denominator accumulator, 2 chunk-halves wide (the two
                # halves are summed exactly in the f32 ones-matmul of norm)
                acc2 = accp.tile([128, 2 * W], BF, tag="acc")
                # Wo slice for this group, loaded up front so the deferred
                # tail (running inside group g+1's loop) has it resident
                wo_g = wop.tile([128, 4, HID], BF, tag="wog")
                nc.sync.dma_start(
                    wo_g[:],
                    woT.ap()[g * 512:(g + 1) * 512, :]
                    .rearrange("(a p) d -> p a d", p=128))

                for r in range(N_CORES):
                    # stream rank r's K^T / V slabs for this head group
                    kt_slab = kvstream.tile([128, LKV], BF, tag="kt")
                    nc.sync.dma_start(
                        kt_slab[:],
                        gathK[g][r * 128 * LKV:(r + 1) * 128 * LKV]
                        .rearrange("(p c) -> p c", p=128))
                    v_slab = kvstream.tile([128, LKV // 128, D], BF, tag="v")
                    nc.sync.dma_start(
                        v_slab[:],
                        gathV[g][r * 128 * LKV:(r + 1) * 128 * LKV]
                        .rearrange("(a p d) -> p a d", p=128, d=D))
                    for l in range(LKV // 128):
                        kc = r * (LKV // 128) + l
                        st = stps.tile([128, W], F32, tag="st")
                        ex = attw.tile([128, W], BF, tag="ex")
                        if kc >= 56:
                            # strip-0 q cols are fully causal-masked for
                            # every core here: compute only the high half
                            # (the mask mul below zeroes the stale half)
                            for hh in range(4):
                                nc.tensor.matmul(
                                    st[:, hh * QS + 128:(hh + 1) * QS],
                                    kt_slab[:, l * 128:(l + 1) * 128],
                                    qt_sb[:, g * 4 + hh, 128:QS],
                                    start=True, stop=True)
                        else:
                            for hh in range(0, 4, 2):
                                nc.tensor.matmul(
                                    st[:, hh * QS:(hh + 2) * QS],
                                    kt_slab[:, l * 128:(l + 1) * 128],
                                    qt_sb[:, g * 4 + hh:g * 4 + hh + 2, :],
                                    start=True, stop=True)
                        if pend:
                            flush()
                        # self-paced background pump (Q projection during
                        # group 0, the previous group's tail afterwards)
                        if bg:
                            ci = g * KC + kc
                            rem = max(1, bg_deadline[0] - ci)
                            n = min(len(bg), -(-len(bg) // rem))
                            for _ in range(n):
                                bg.pop(0)()
                        if kc >= 56:
                            nc.scalar.activation(
                                ex.rearrange(
                                    "p (h q) -> p h q", q=QS)[:, :, 128:],
                                st[:].rearrange(
                                    "p (h q) -> p h q", q=QS)[:, :, 128:],
                                mybir.ActivationFunctionType.Exp,
                                scale=SCALE)
                        else:
                            nc.scalar.activation(
                                ex[:], st[:],
                                mybir.ActivationFunctionType.Exp,
                                scale=SCALE)
                        if kc >= BND:
                            nc.vector.tensor_mul(
                                ex[:], ex[:], mask_sb[:, kc - BND, :])
                        pend.append((ex, l, v_slab, kc, g, ot_ps, acc2,
                                     wo_g))
            flush()
            while bg:
                bg.pop(0)()

        # ---------------- Phase F: store the accumulated output -----------
        for j4 in range(4):
            nc.sync.dma_start(
                outT.ap()[j4 * 512:(j4 + 1) * 512, :]
                .rearrange("(a p) q -> p a q", p=128),
                out_acc[:, j4 * 4:(j4 + 1) * 4, :])


def _get_nc():
    if "nc" not in _CACHE:
        _CACHE["nc"] = _build()
    return _CACHE["nc"]


def _make_in_maps(x_q, x_kv, Wq, Wk, Wv, Wo):
    xqT_full = np.ascontiguousarray(x_q[0].T)           # [HID, SL]
    xkvT_full = np.ascontiguousarray(x_kv[0].T)         # [HID, SKV]
    wqT = np.ascontiguousarray(Wq.T).astype(BF16)
    wkT = np.ascontiguousarray(Wk.T).astype(BF16)
    wvT = np.ascontiguousarray(Wv.T).astype(BF16)
    woT = np.ascontiguousarray(Wo.T).astype(BF16)

    in_maps = []
    for c in range(N_CORES):
        s0, s1 = c, 15 - c
        xqT = np.concatenate(
            [xqT_full[:, s0 * 128:(s0 + 1) * 128],
             xqT_full[:, s1 * 128:(s1 + 1) * 128]], axis=1).astype(BF16)
        xkvT = np.ascontiguousarray(
            xkvT_full[:, c * LKV:(c + 1) * LKV]).astype(BF16)
        # causal masks for key chunks 48..63, replicated across the 4 heads
        # of a kv group (so one tensor_mul covers [128, 4*QS])
        mask = np.zeros((16, 128, QS), dtype=np.float32)
        kk = np.arange(128)
        for j in range(16):
            key_g = (BND + j) * 128 + kk                # [128]
            for half, st in enumerate((s0, s1)):
                q_g = RANK_OFF + st * 128 + np.arange(128)   # [128]
                mask[j, :, half * 128:(half + 1) * 128] = (
                    key_g[:, None] <= q_g[None, :])
        mask4 = np.tile(mask, (1, 1, 4))                # [16, 128, 4*QS]
        in_maps.append({
            "xqT": xqT, "xkvT": xkvT, "wqT": wqT, "wkT": wkT,
            "wvT": wvT, "woT": woT,
            "mask": mask4.reshape(16 * 128, 4 * QS).astype(BF16),
        })
    return in_maps


def _unshard(results):
    out = np.empty((1, SL, HID), dtype=np.float32)
    for c in range(N_CORES):
        outT = results[c]["outT"]                       # [HID, QS]
        s0, s1 = c, 15 - c
        out[0, s0 * 128:(s0 + 1) * 128, :] = outT[:, 0:128].T
        out[0, s1 * 128:(s1 + 1) * 128, :] = outT[:, 128:256].T
    return out


def kernel(x_q, x_kv, Wq, Wk, Wv, Wo, _trace=False, _result_box=None):
    nc = _get_nc()
    in_maps = _make_in_maps(x_q, x_kv, Wq, Wk, Wv, Wo)
    res = bass_utils.run_bass_kernel_spmd(
        nc, in_maps, core_ids=list(range(N_CORES)), trace=_trace)
    if _result_box is not None:
        _result_box.append(res)
    return _unshard(res.results)
